# revision 6
# baseline (speedup 1.0000x reference)
"""Overlapping-chunk extraction kernel for Trainium2 (Bass).

Computes out[b, j, c, f] = x[b, 125*j + c, f] for j in [0, 255), c in [0, 250),
i.e. 255 half-overlapping chunks of length 250 from a (16, 32000, 64) signal.

Strategy (pure data movement, memory-bound):
  - Shard batch across 8 cores: 2 samples per core.
  - Per sample: ONE direct HBM->HBM DMA. Source = overlapping strided view
    (255 blocks of 16000 fp32 at stride 8000). Destination = fully contiguous
    output sample. The two per-sample DMAs run concurrently on the gpsimd
    (SWDGE) queue.

Measured ~195-205 us/core (free-running repeat differencing at R=201,
8-core SPMD) = 92-94% of the 182 us HBM-per-NC floor for the 65.3 MB/core
of HBM traffic. Barrier-chained repeat benches read ~60-90 us/iter higher
because each barrier forces Q7 to re-emit all 510 descriptors serially;
a single-shot invocation overlaps emission with transfer and runs at the
free-running rate. Alternatives measured slower on this axon/PJRT runtime:
  - contiguous-read/strided-write HBM->HBM mirror: +25%.
  - SBUF staging (cuts HBM traffic 65->49 MB/core): 6x slower — HBM->SBUF
    loads run at ~42 GB/s on gpsimd (~1.5 us/descriptor regardless of
    size), ~114 GB/s on sync/HWDGE; strided SBUF->HBM stores ~23 GB/s.
    Even the best correct staged pipeline (HWDGE loads + ACT-engine
    on-chip duplication so stores are a fully contiguous DRAM stream at
    215 GB/s in isolation) measured 1.6 ms — component speeds do not
    compose once loads/copies/stores interleave (see variants.py v3a).
  - serializing the two DMAs: +30%; splitting into 4-8 DMAs: ~2x;
    single fused 3-dim DMA: +28%; HWDGE (sync/scalar) queues: ~2x;
    spreading across 2-3 queues: 1.3-4x.
  - fp32->bf16 cast on the output leg (halves write bytes to the minimal
    49 MB/core, rel err 3e-3 vs the 2e-2 gate): same speed as fp32 — the
    read/engine path binds, not write volume. Kept exact fp32.
  - static DMAs (mybir.InstLoad): unsupported by this walrus pass
    pipeline (no static-ring allocation; lowers via generateDynamicDMA).
"""

import numpy as np

import concourse.bass as bass
import concourse.mybir as mybir
from concourse.bass_utils import run_bass_kernel_spmd

# Problem shape (hardcoded per contract)
B, T, F = 16, 32000, 64
N_CORES = 8
S = B // N_CORES          # samples per core = 2
NFC = 128                 # non-overlapping chunks per sample
CHUNK = 250               # frames per chunk
NOV = 2 * NFC - 1         # 255 overlapped output chunks
PART_FREE = CHUNK * F     # 16000 fp32 per chunk
HALF_FREE = PART_FREE // 2  # 8000 fp32 = 125 frames (chunk advance)
SAMPLE_IN = T * F         # 2_048_000 fp32 per input sample
SAMPLE_OUT = NOV * PART_FREE  # 4_080_000 fp32 per output sample

_NC_CACHE = {}


def _build_module():
    nc = bass.Bass(trn_type="TRN2")
    x = nc.dram_tensor("x", [S, T, F], mybir.dt.float32, kind="ExternalInput")
    y = nc.dram_tensor(
        "y", [S, NOV, CHUNK, F], mybir.dt.float32, kind="ExternalOutput"
    )
    x_t = x[:, :, :].tensor
    y_t = y[:, :, :, :].tensor

    with (
        nc.semaphore("st") as st,
        nc.Block() as block,
    ):
        @block.gpsimd
        def _(gpsimd):
            with nc.allow_non_contiguous_dma(reason="overlapping chunk reads"):
                for s in range(S):
                    src = bass.AP(
                        x_t, s * SAMPLE_IN, [[HALF_FREE, NOV], [1, PART_FREE]]
                    )
                    dst = bass.AP(
                        y_t, s * SAMPLE_OUT, [[PART_FREE, NOV], [1, PART_FREE]]
                    )
                    gpsimd.dma_start(dst, src).then_inc(st, 16)
                gpsimd.wait_ge(st, 16 * S)

    return nc


def get_module():
    if "nc" not in _NC_CACHE:
        _NC_CACHE["nc"] = _build_module()
    return _NC_CACHE["nc"]


def kernel(x):
    x = np.ascontiguousarray(np.asarray(x), dtype=np.float32)
    assert x.shape == (B, T, F), x.shape
    nc = get_module()
    in_maps = [{"x": x[i * S : (i + 1) * S]} for i in range(N_CORES)]
    res = run_bass_kernel_spmd(nc, in_maps, core_ids=list(range(N_CORES)))
    return np.concatenate([r["y"] for r in res.results], axis=0)



# revision 8
# speedup vs baseline: 1.2136x; 1.2136x over previous
"""Overlapping-chunk extraction kernel for Trainium2 (Bass) — bf16 transport.

Computes out[b, j, c, f] = x[b, 125*j + c, f] for j in [0, 255), c in [0, 250),
i.e. 255 half-overlapping chunks of length 250 from a (16, 32000, 64) signal.

Strategy (pure data movement, memory-bound):
  - Shard batch across 8 cores: 2 samples per core.
  - Host pre-casts x to bf16. Elementwise this is identical to casting the
    gathered output (the op only copies elements), so accuracy is bf16
    rounding: measured 2.9e-3 max rel err vs the 2e-2 gate (7x margin).
  - Per sample: ONE direct HBM->HBM bf16 DMA. Source = overlapping strided
    view (255 blocks of 16000 elems at stride 8000); destination = fully
    contiguous output sample. The two per-sample DMAs run concurrently on
    the gpsimd (SWDGE) queue. Host upcasts the result to fp32.
  - bf16 halves HBM traffic vs fp32 (65.3 -> 32.6 MB/core). Same-window
    interleaved A/B measured 1.85x faster than the fp32 version
    (141-157 us vs 260-294 us per core); earlier cross-window comparisons
    that suggested "no gain" were ambient-drift artifacts.

Other measured-slower alternatives on this axon/PJRT runtime (see
variants.py): SBUF staging in any form (loads ~42 GB/s on gpsimd, strided
SBUF stores ~23 GB/s, staged+duplication pipeline 1.6 ms), HWDGE
(sync/scalar) queues (~2x or crash), multi-queue splits (1.3-4x),
serializing or further splitting the DMAs (+30% to 2x), fp32->bf16 cast
inside the DMA (cast datapath eats most of the gain), static DMAs
(unsupported by this walrus pass pipeline).
"""

import numpy as np
import ml_dtypes

import concourse.bass as bass
import concourse.mybir as mybir
from concourse.bass_utils import run_bass_kernel_spmd

# Problem shape (hardcoded per contract)
B, T, F = 16, 32000, 64
N_CORES = 8
S = B // N_CORES          # samples per core = 2
NFC = 128                 # non-overlapping chunks per sample
CHUNK = 250               # frames per chunk
NOV = 2 * NFC - 1         # 255 overlapped output chunks
PART_FREE = CHUNK * F     # 16000 elems per chunk
HALF_FREE = PART_FREE // 2  # 8000 elems = 125 frames (chunk advance)
SAMPLE_IN = T * F         # 2_048_000 elems per input sample
SAMPLE_OUT = NOV * PART_FREE  # 4_080_000 elems per output sample

_NC_CACHE = {}


def _build_module():
    nc = bass.Bass(trn_type="TRN2")
    x = nc.dram_tensor("x", [S, T, F], mybir.dt.bfloat16, kind="ExternalInput")
    y = nc.dram_tensor(
        "y", [S, NOV, CHUNK, F], mybir.dt.bfloat16, kind="ExternalOutput"
    )
    x_t = x[:, :, :].tensor
    y_t = y[:, :, :, :].tensor

    with (
        nc.semaphore("st") as st,
        nc.Block() as block,
    ):
        @block.gpsimd
        def _(gpsimd):
            with nc.allow_non_contiguous_dma(reason="overlapping chunk reads"):
                for s in range(S):
                    src = bass.AP(
                        x_t, s * SAMPLE_IN, [[HALF_FREE, NOV], [1, PART_FREE]]
                    )
                    dst = bass.AP(
                        y_t, s * SAMPLE_OUT, [[PART_FREE, NOV], [1, PART_FREE]]
                    )
                    gpsimd.dma_start(dst, src).then_inc(st, 16)
                gpsimd.wait_ge(st, 16 * S)

    return nc


def get_module():
    if "nc" not in _NC_CACHE:
        _NC_CACHE["nc"] = _build_module()
    return _NC_CACHE["nc"]


def kernel(x):
    x = np.asarray(x, dtype=np.float32).astype(ml_dtypes.bfloat16)
    assert x.shape == (B, T, F), x.shape
    nc = get_module()
    in_maps = [{"x": x[i * S : (i + 1) * S]} for i in range(N_CORES)]
    res = run_bass_kernel_spmd(nc, in_maps, core_ids=list(range(N_CORES)))
    return np.concatenate(
        [np.asarray(r["y"]).astype(np.float32) for r in res.results], axis=0
    )
